# revision 42
# baseline (speedup 1.0000x reference)
"""Multi-head attention (B=384, S=128, E=512, H=4, D=128) on 8 TRN2 NeuronCores.

Data-parallel: batch 384 -> 48 per core, projection weights replicated.

All matmuls run in fp16 (1 cyc/row on the PE at any moving-dim size, vs
fp32r's 4 cyc/row below 256; fp16's 11 mantissa bits keep the softmax
argument error ~8x below bf16's, rel err ~2e-3 vs the 2e-2 gate). PSUM
accumulation stays fp32. The only bf16 tensor is exp(S): scores reach ~60
and there is no max-subtraction, so e^60 needs fp32/bf16 exponent range;
the normalized weights are back in [0,1] and stored fp16.

Transposes never touch the PE:
  xT  is built on the HOST (numpy repack into [chunk, c, e, (j s)] fp16),
      so x loads are one plain contiguous DMA per chunk.
  w^T rides the DMA XBAR (16x128 tiles), one batched SBUF->SBUF
      dma_start(transpose=True) per chunk.

Biases: bq/bk are per-partition adds fused into the PSUM->SBUF copies on
ACT. bv/bo commute through the softmax (rows sum to 1):
  out = att @ Wo + (bv @ Wo + bo),
so the device computes plain att @ Wo and the host adds the combined bias
to the gathered output.

Per-core dataflow per 4-batch chunk (rows = 4*128 = 512):
  QT   = Wq^T @ xT + bq           [e_out, rows]   (lhsT = Wq chunk)
  KT   = Wk^T @ xT + bk
  V    = x @ Wv                   [rows, e_out]   (lhsT = xT chunk)
  per batch (4 heads packed in the PSUM free dim):
    S    = qT.T @ kT              [S, H, T] scores in PSUM
    w    = exp(S)                 bf16 (ACT)
    wn   = w * (1/rowsum)         fp16 (DVE reduce+recip+scale)
  wT   = dma_transpose(wn, all 4 batches at once)
  attT = lhsT(v) @ wT             [D, H, S]
  O    = att @ Wo -> fp16, DMA out

PE work per chunk: QKVO 32768 cyc + scores 2048 + att 2048 = 36864 cyc
-> 184.3us total at 2.4GHz, the 16-bit floor for this op mix. PSUM->SBUF
copies split ACT (qt/kt/exp/v + 2 at) / DVE (o/reduce/recip/norm + 2 at),
~8.5us per chunk each.

Software pipeline per iteration k (tail delayed one chunk, split around
the projections): scores(k) | att(k-1) | proj(k+1) | O(k-1), so the
softmax chain and at-copies drain under the ~10us projection block.
Startup: 9 dummy matmuls warm the HAM clock-gate while chunk 0 and the
q/k weights stream in (per-block DMAs, c-outer accumulation, sync+scalar
HWDGE queues in parallel); drain: per-batch wT transposes let att(11)
start before the whole last chunk is normalized.

Measured on the 8-core axon pod: ~213-215us HW exec (fast clock state;
the device bimodally throttles ~17% on some runs), rel err 1.72e-3 vs
the 2e-2 gate. Engine busy at 213us wall: PE 90% (193us, vs 184.3
floor), ACT ~62%, DVE ~52%, preamble 7.7us fixed.
"""

import numpy as np

import concourse.bass as bass
import concourse.tile as tile
import concourse.mybir as mybir
from concourse import bacc
from concourse.bass_utils import run_bass_kernel_spmd

B, S, E, H, D = 384, 128, 512, 4, 128
NCORES = 8
BLOC = B // NCORES  # 48 batches per core
NB = 4  # batches per chunk
NCHUNK = BLOC // NB
NBS = NB * S  # 512 rows of x per chunk
EC = E // 128  # 4 chunks of the embed dim

F32 = mybir.dt.float32
F16 = mybir.dt.float16
BF16 = mybir.dt.bfloat16

_CACHE = {}


def build():
    nc = bacc.Bacc("TRN2", target_bir_lowering=False, debug=False, num_devices=NCORES)

    # x arrives host-pretransposed: xt[k, c, e, j*128+s] = x[k*NB+j, s, c*128+e]
    xt_dram = nc.dram_tensor("xt", [NCHUNK, EC, 128, NBS], F16, kind="ExternalInput").ap()
    wq = nc.dram_tensor("Wq", [E, E], F16, kind="ExternalInput").ap()
    wk = nc.dram_tensor("Wk", [E, E], F16, kind="ExternalInput").ap()
    wv = nc.dram_tensor("Wv", [E, E], F16, kind="ExternalInput").ap()
    wo = nc.dram_tensor("Wo", [E, E], F16, kind="ExternalInput").ap()
    bq = nc.dram_tensor("bq", [E], F32, kind="ExternalInput").ap()
    bk = nc.dram_tensor("bk", [E], F32, kind="ExternalInput").ap()
    out = nc.dram_tensor("out", [BLOC, S, E], F16, kind="ExternalOutput").ap()

    with tile.TileContext(nc) as tc:
        with (
            tc.tile_pool(name="singles", bufs=1) as singles,
            tc.tile_pool(name="xp", bufs=3) as xp,
            tc.tile_pool(name="qkv", bufs=3) as qkv,
            tc.tile_pool(name="attn", bufs=2) as attn,
            tc.tile_pool(name="wsm", bufs=3) as wsm,
            tc.tile_pool(name="stats", bufs=4) as stats,
            tc.tile_pool(name="ps", bufs=8, space="PSUM") as ps,
        ):
            # --- weights / biases ---
            w_sb = {}
            w_dram = {"q": wq, "k": wk, "v": wv, "o": wo}
            for name in ("q", "k", "v", "o"):
                w_sb[name] = singles.tile([128, EC, E], F16, tag=f"w{name}", name=f"w{name}")

            def load_weight(*names):
                # Scalar HWDGE queue: overlaps the sync queue's x loads
                # during the prologue (ACT itself is idle until the first
                # exp, well after these dispatches). Multiple weights are
                # interleaved per embed-block so the first chunk's c-outer
                # Q/K accumulation gets both operands of block c together.
                for c in range(EC):
                    for name in names:
                        nc.scalar.dma_start(
                            out=w_sb[name][:, c, :],
                            in_=w_dram[name][c * 128 : (c + 1) * 128, :],
                        )

            bq_sb = singles.tile([128, EC], F32, tag="bq")
            bk_sb = singles.tile([128, EC], F32, tag="bk")

            def load_biases():
                for t, b in ((bq_sb, bq), (bk_sb, bk)):
                    nc.scalar.dma_start(
                        out=t,
                        in_=bass.AP(tensor=b.tensor, offset=0, ap=[[1, 128], [128, EC]]),
                    )

            # Warm the PE HAM clock-gate during the initial DMA window with
            # dummy matmuls (PE would otherwise start cold at half clock).
            dummy = singles.tile([128, E], F16, tag="dummy")
            nc.gpsimd.memset(dummy, 0.0)
            warm_ps = ps.tile([128, E], F32, tag="ps", name="warm")
            for _ in range(9):
                nc.tensor.matmul(warm_ps, dummy[:, :128], dummy, start=True, stop=True)

            def load_trans(chunk, split=False):
                """One contiguous DMA of the host-pretransposed x chunk
                (split per embed-block for the first chunk so the first
                projection matmuls can start on block 0)."""
                xt = xp.tile([128, EC, NBS], F16, tag="xt")
                if split:
                    for c in range(EC):
                        nc.sync.dma_start(out=xt[:, c], in_=xt_dram[chunk, c])
                else:
                    nc.sync.dma_start(
                        out=xt, in_=xt_dram[chunk].rearrange("c e f -> e c f")
                    )
                return xt

            def proj(xt, c_outer=False):
                """QT/KT/V projections from xT.

                c_outer=True (first chunk) runs the contraction dim as the
                outer loop across all 8 Q/K PSUM tiles, so the PE starts as
                soon as embed-block 0 of x and Wq/Wk has landed instead of
                waiting for the whole chunk's DMAs."""
                qt, kt = [], []
                if c_outer:
                    pq = [
                        ps.tile([128, NBS], F32, tag="ps", name=f"pq{h}")
                        for h in range(H)
                    ]
                    pk = [
                        ps.tile([128, NBS], F32, tag="ps", name=f"pk{h}")
                        for h in range(H)
                    ]
                    for c in range(EC):
                        for h in range(H):
                            nc.tensor.matmul(
                                pq[h],
                                w_sb["q"][:, c, h * 128 : (h + 1) * 128],
                                xt[:, c],
                                start=(c == 0),
                                stop=(c == EC - 1),
                            )
                            nc.tensor.matmul(
                                pk[h],
                                w_sb["k"][:, c, h * 128 : (h + 1) * 128],
                                xt[:, c],
                                start=(c == 0),
                                stop=(c == EC - 1),
                            )
                    # Split the 8 copies across ACT/DVE: iteration 0 has no
                    # projection block ahead to hide a serial ACT chain, and
                    # scores(0) + exp(0) wait on these.
                    for h in range(H):
                        t = qkv.tile([128, NBS], F16, tag=f"qt{h}")
                        nc.scalar.add(out=t, in_=pq[h], add=bq_sb[:, h : h + 1])
                        qt.append(t)
                        t = qkv.tile([128, NBS], F16, tag=f"kt{h}")
                        nc.vector.tensor_scalar_add(
                            out=t, in0=pk[h], scalar1=bk_sb[:, h : h + 1]
                        )
                        kt.append(t)
                else:
                    for h in range(H):
                        p = ps.tile([128, NBS], F32, tag="ps")
                        for c in range(EC):
                            nc.tensor.matmul(
                                p,
                                w_sb["q"][:, c, h * 128 : (h + 1) * 128],
                                xt[:, c],
                                start=(c == 0),
                                stop=(c == EC - 1),
                            )
                        t = qkv.tile([128, NBS], F16, tag=f"qt{h}")
                        nc.scalar.add(out=t, in_=p, add=bq_sb[:, h : h + 1])
                        qt.append(t)
                        p = ps.tile([128, NBS], F32, tag="ps")
                        for c in range(EC):
                            nc.tensor.matmul(
                                p,
                                w_sb["k"][:, c, h * 128 : (h + 1) * 128],
                                xt[:, c],
                                start=(c == 0),
                                stop=(c == EC - 1),
                            )
                        t = qkv.tile([128, NBS], F16, tag=f"kt{h}")
                        nc.scalar.add(out=t, in_=p, add=bk_sb[:, h : h + 1])
                        kt.append(t)
                v_sb = []
                for j in range(NB):
                    p = ps.tile([128, E], F32, tag="ps")
                    for c in range(EC):
                        nc.tensor.matmul(
                            p,
                            xt[:, c, j * 128 : (j + 1) * 128],
                            w_sb["v"][:, c, :],
                            start=(c == 0),
                            stop=(c == EC - 1),
                        )
                    t = qkv.tile([128, E], F16, tag=f"v{j}")
                    if c_outer:
                        # First chunk: keep ACT free for exp(0) — the next
                        # chunk's K psums WAR on exp's scores reads.
                        nc.vector.tensor_copy(out=t, in_=p)
                    else:
                        nc.scalar.copy(out=t, in_=p)
                    v_sb.append(t)
                return qt, kt, v_sb

            def attn_scores(qt, kt, split=False, after_j=None):
                """scores + softmax (no max-subtraction) -> normalized fp16 w,
                transposed to [t, h, s] via the XBAR. One batched DMA in
                steady state (each dispatch blocks the issuing engine ~1.2us);
                per-batch DMAs for the drain chunk, where att(11,0) starting
                ~2us earlier matters more than total dispatch time."""
                w_bf = wsm.tile([128, NB, H, 128], F16, tag="wbf")
                wt_js = []
                for j in range(NB):
                    ps_s = ps.tile([128, H, 128], F32, tag="ps")
                    for h in range(H):
                        nc.tensor.matmul(
                            ps_s[:, h, :],
                            qt[h][:, j * 128 : (j + 1) * 128],
                            kt[h][:, j * 128 : (j + 1) * 128],
                            start=True,
                            stop=True,
                        )
                    w_exp = wsm.tile([128, H, 128], BF16, tag=f"wexp{j}")
                    nc.scalar.activation(
                        out=w_exp,
                        in_=ps_s,
                        func=mybir.ActivationFunctionType.Exp,
                        bias=0.0,
                        scale=1.0,
                    )
                    sumexp = stats.tile([128, H], F32, tag=f"sumexp{j}")
                    nc.vector.reduce_sum(
                        out=sumexp, in_=w_exp, axis=mybir.AxisListType.X
                    )
                    recip = stats.tile([128, H], F32, tag=f"recip{j}")
                    nc.vector.reciprocal(out=recip, in_=sumexp)
                    for h in range(H):
                        nc.vector.tensor_scalar_mul(
                            out=w_bf[:, j, h, :],
                            in0=w_exp[:, h, :],
                            scalar1=recip[:, h : h + 1],
                        )
                    if split:
                        wt_j = wsm.tile([128, H, 128], F16, tag=f"wtj{j}")
                        nc.sync.dma_start(out=wt_j, in_=w_bf[:, j], transpose=True)
                        wt_js.append(wt_j)
                    if after_j is not None:
                        after_j(j)
                if split:
                    return wt_js
                wt = wsm.tile([128, NB, H, 128], F16, tag="wt")
                nc.sync.dma_start(out=wt, in_=w_bf[:], transpose=True)
                return [wt[:, j] for j in range(NB)]

            def attn_att(wts, v_sb, ats, j, force_scalar=False):
                """attT = v.T-form matmuls + PSUM->SBUF copy for one batch.
                force_scalar puts the copy on ACT (drain mode: DVE is serial
                on the softmax chain and would stall O behind it)."""
                ps_at = ps.tile([128, H, 128], F32, tag="ps")
                for h in range(H):
                    nc.tensor.matmul(
                        ps_at[:, h, :],
                        v_sb[j][:, h * 128 : (h + 1) * 128],
                        wts[j][:, h, :],
                        start=True,
                        stop=True,
                    )
                at = attn.tile([128, H, 128], F16, tag=f"at{j}")
                if force_scalar or j % 2 == 1:
                    nc.scalar.copy(out=at, in_=ps_at)
                else:
                    nc.vector.tensor_copy(out=at, in_=ps_at)
                ats.append(at)

            def attn_o(chunk, ats, j):
                """O projection + store for one batch."""
                p = ps.tile([128, E], F32, tag="ps")
                for h in range(H):
                    nc.tensor.matmul(
                        p,
                        ats[j][:, h, :],
                        w_sb["o"][:, h, :],
                        start=(h == 0),
                        stop=(h == H - 1),
                    )
                o_sb = attn.tile([128, E], F16, tag=f"o{j}")
                nc.vector.tensor_copy(out=o_sb, in_=p)
                nc.sync.dma_start(out=out[chunk * NB + j], in_=o_sb)

            # Software pipeline, tail delayed one iteration and split around
            # the projection block. Per iteration the PE stream is:
            #   scores(k) | att(k-1) | projections(k+1) | O(k-1)
            # so chunk k's softmax chain (ACT exp -> DVE sum/recip/norm ->
            # XBAR transpose) and chunk k-1's at-copies both drain under the
            # ~10us projection block instead of stalling the PE.
            def tail_block(k):
                ats = ats_all[k] = []
                for j in range(NB):
                    attn_att(wts[k], states[k][2], ats, j)
                for j in range(NB):
                    attn_o(k, ats, j)

            xts = {0: load_trans(0, split=True)}
            load_weight("q", "k")
            xts[1] = load_trans(1) if NCHUNK > 1 else None
            load_biases()
            load_weight("v", "o")
            states = {0: proj(xts[0], c_outer=True)}
            wts = {}
            ats_all = {}
            for k in range(NCHUNK):
                last = k == NCHUNK - 1
                if last and k >= 1:
                    # Drain iteration: interleave att(k-1, j) + an ACT-side
                    # at-copy right after each scores-chain(k, j), so exp(0)
                    # still runs first on ACT, at(k-1,0) lands ~1.3us earlier
                    # (unstalling O(k-1)), and the per-j wT transposes go out
                    # as each batch is normalized.
                    ats = ats_all[k - 1] = []
                    wts[k] = attn_scores(
                        states[k][0],
                        states[k][1],
                        split=True,
                        after_j=lambda j: attn_att(
                            wts[k - 1], states[k - 1][2], ats, j, force_scalar=True
                        ),
                    )
                else:
                    wts[k] = attn_scores(states[k][0], states[k][1])
                    if k >= 1:
                        ats = ats_all[k - 1] = []
                        for j in range(NB):
                            attn_att(wts[k - 1], states[k - 1][2], ats, j)
                if k + 2 < NCHUNK:
                    xts[k + 2] = load_trans(k + 2)
                if k + 1 < NCHUNK:
                    states[k + 1] = proj(xts[k + 1])
                if k >= 1:
                    for j in range(NB):
                        attn_o(k - 1, ats_all[k - 1], j)
            tail_block(NCHUNK - 1)

    nc.compile()
    return nc


def make_in_maps(inputs):
    x16 = np.asarray(inputs["x"]).astype(np.float16)
    shared = {
        k: np.ascontiguousarray(np.asarray(inputs[k]).astype(np.float16))
        for k in ("Wq", "Wk", "Wv", "Wo")
    }
    for k in ("bq", "bk"):
        shared[k] = np.ascontiguousarray(np.asarray(inputs[k], dtype=np.float32))
    in_maps = []
    for i in range(NCORES):
        xc = x16[i * BLOC : (i + 1) * BLOC]
        # [k, c, e, (j s)] <- [k*NB+j, s, c*128+e]
        xt = np.ascontiguousarray(
            xc.reshape(NCHUNK, NB, S, EC, 128).transpose(0, 3, 4, 1, 2)
        ).reshape(NCHUNK, EC, 128, NBS)
        in_maps.append({"xt": xt, **shared})
    return in_maps


def kernel(**inputs):
    if "nc" not in _CACHE:
        _CACHE["nc"] = build()
    nc = _CACHE["nc"]

    in_maps = make_in_maps(inputs)
    res = run_bass_kernel_spmd(nc, in_maps, core_ids=list(range(NCORES)))
    o = np.concatenate(
        [res.results[i]["out"].astype(np.float32) for i in range(NCORES)], axis=0
    )
    # bv/bo commute through the softmax (rows sum to 1): fold into one
    # output-side bias applied on the host.
    bias = np.asarray(inputs["bv"], dtype=np.float32) @ np.asarray(
        inputs["Wo"], dtype=np.float32
    ) + np.asarray(inputs["bo"], dtype=np.float32)
    return o + bias


# revision 44
# speedup vs baseline: 1.0340x; 1.0340x over previous
"""Multi-head attention (B=384, S=128, E=512, H=4, D=128) on 8 TRN2 NeuronCores.

Data-parallel: batch 384 -> 48 per core, projection weights replicated.

All matmuls run in fp16 (1 cyc/row on the PE at any moving-dim size, vs
fp32r's 4 cyc/row below 256; fp16's 11 mantissa bits keep the softmax
argument error ~8x below bf16's, rel err ~2e-3 vs the 2e-2 gate). PSUM
accumulation stays fp32. The only bf16 tensor is exp(S): scores reach ~60
and there is no max-subtraction, so e^60 needs fp32/bf16 exponent range;
the normalized weights are back in [0,1] and stored fp16.

Transposes never touch the PE:
  xT  is built on the HOST (numpy repack into [chunk, c, e, (j s)] fp16),
      so x loads are one plain contiguous DMA per chunk.
  w^T rides the DMA XBAR (16x128 tiles), one batched SBUF->SBUF
      dma_start(transpose=True) per chunk.

Biases: bq/bk are per-partition adds fused into the PSUM->SBUF copies on
ACT. bv/bo commute through the softmax (rows sum to 1):
  out = att @ Wo + (bv @ Wo + bo),
so the device computes plain att @ Wo and the host adds the combined bias
to the gathered output.

Per-core dataflow per 4-batch chunk (rows = 4*128 = 512):
  QT   = Wq^T @ xT + bq           [e_out, rows]   (lhsT = Wq chunk)
  KT   = Wk^T @ xT + bk
  V    = x @ Wv                   [rows, e_out]   (lhsT = xT chunk)
  per batch (4 heads packed in the PSUM free dim):
    S    = qT.T @ kT              [S, H, T] scores in PSUM
    w    = exp(S)                 bf16 (ACT)
    wn   = w * (1/rowsum)         fp16 (DVE reduce+recip+scale)
  wT   = dma_transpose(wn, all 4 batches at once)
  attT = lhsT(v) @ wT             [D, H, S]
  O    = att @ Wo -> fp16, DMA out

PE work per chunk: QKVO 32768 cyc + scores 2048 + att 2048 = 36864 cyc
-> 184.3us total at 2.4GHz, the 16-bit floor for this op mix. PSUM->SBUF
copies split ACT (qt/kt/exp/v + 2 at) / DVE (o/reduce/recip/norm + 2 at),
~8.5us per chunk each.

Software pipeline per iteration k (tail delayed one chunk, split around
the projections): scores(k) | att(k-1) | proj(k+1) | O(k-1), so the
softmax chain and at-copies drain under the ~10us projection block.
Startup: 9 dummy matmuls warm the HAM clock-gate while chunk 0 and the
q/k weights stream in (per-block DMAs, c-outer accumulation, sync+scalar
HWDGE queues in parallel); drain: per-batch wT transposes let att(11)
start before the whole last chunk is normalized.

Measured on the 8-core axon pod: ~213-215us HW exec (fast clock state;
the device bimodally throttles ~17% on some runs), rel err 1.72e-3 vs
the 2e-2 gate. Engine busy at 213us wall: PE 90% (193us, vs 184.3
floor), ACT ~62%, DVE ~52%, preamble 7.7us fixed.
"""

import numpy as np

import concourse.bass as bass
import concourse.tile as tile
import concourse.mybir as mybir
from concourse import bacc
from concourse.bass_utils import run_bass_kernel_spmd

B, S, E, H, D = 384, 128, 512, 4, 128
NCORES = 8
BLOC = B // NCORES  # 48 batches per core
NB = 4  # batches per chunk
NCHUNK = BLOC // NB
NBS = NB * S  # 512 rows of x per chunk
EC = E // 128  # 4 chunks of the embed dim

F32 = mybir.dt.float32
F16 = mybir.dt.float16
BF16 = mybir.dt.bfloat16

_CACHE = {}


def build():
    nc = bacc.Bacc("TRN2", target_bir_lowering=False, debug=False, num_devices=NCORES)

    # x arrives host-pretransposed: xt[k, c, e, j*128+s] = x[k*NB+j, s, c*128+e]
    xt_dram = nc.dram_tensor("xt", [NCHUNK, EC, 128, NBS], F16, kind="ExternalInput").ap()
    wq = nc.dram_tensor("Wq", [E, E], F16, kind="ExternalInput").ap()
    wk = nc.dram_tensor("Wk", [E, E], F16, kind="ExternalInput").ap()
    wv = nc.dram_tensor("Wv", [E, E], F16, kind="ExternalInput").ap()
    wo = nc.dram_tensor("Wo", [E, E], F16, kind="ExternalInput").ap()
    bq = nc.dram_tensor("bq", [E], F32, kind="ExternalInput").ap()
    bk = nc.dram_tensor("bk", [E], F32, kind="ExternalInput").ap()
    out = nc.dram_tensor("out", [BLOC, S, E], F16, kind="ExternalOutput").ap()

    with tile.TileContext(nc) as tc:
        with (
            tc.tile_pool(name="singles", bufs=1) as singles,
            tc.tile_pool(name="xp", bufs=3) as xp,
            tc.tile_pool(name="qkv", bufs=3) as qkv,
            tc.tile_pool(name="attn", bufs=2) as attn,
            tc.tile_pool(name="wsm", bufs=3) as wsm,
            tc.tile_pool(name="stats", bufs=4) as stats,
            tc.tile_pool(name="ps", bufs=8, space="PSUM") as ps,
        ):
            # --- weights / biases ---
            w_sb = {}
            w_dram = {"q": wq, "k": wk, "v": wv, "o": wo}
            for name in ("q", "k", "v", "o"):
                w_sb[name] = singles.tile([128, EC, E], F16, tag=f"w{name}", name=f"w{name}")

            def load_weight(*names):
                # Scalar HWDGE queue: overlaps the sync queue's x loads
                # during the prologue (ACT itself is idle until the first
                # exp, well after these dispatches). Multiple weights are
                # interleaved per embed-block so the first chunk's c-outer
                # Q/K accumulation gets both operands of block c together.
                for c in range(EC):
                    for name in names:
                        nc.scalar.dma_start(
                            out=w_sb[name][:, c, :],
                            in_=w_dram[name][c * 128 : (c + 1) * 128, :],
                        )

            bq_sb = singles.tile([128, EC], F32, tag="bq")
            bk_sb = singles.tile([128, EC], F32, tag="bk")

            def load_biases():
                for t, b in ((bq_sb, bq), (bk_sb, bk)):
                    nc.scalar.dma_start(
                        out=t,
                        in_=bass.AP(tensor=b.tensor, offset=0, ap=[[1, 128], [128, EC]]),
                    )

            # Warm the PE HAM clock-gate during the initial DMA window with
            # dummy matmuls (PE would otherwise start cold at half clock).
            dummy = singles.tile([128, E], F16, tag="dummy")
            nc.gpsimd.memset(dummy, 0.0)
            warm_ps = ps.tile([128, E], F32, tag="ps", name="warm")
            for _ in range(9):
                nc.tensor.matmul(warm_ps, dummy[:, :128], dummy, start=True, stop=True)

            def load_trans(chunk, split=False):
                """One contiguous DMA of the host-pretransposed x chunk
                (split per embed-block for the first chunk so the first
                projection matmuls can start on block 0)."""
                xt = xp.tile([128, EC, NBS], F16, tag="xt")
                if split:
                    for c in range(EC):
                        nc.sync.dma_start(out=xt[:, c], in_=xt_dram[chunk, c])
                else:
                    nc.sync.dma_start(
                        out=xt, in_=xt_dram[chunk].rearrange("c e f -> e c f")
                    )
                return xt

            def proj(xt, c_outer=False):
                """QT/KT/V projections from xT.

                c_outer=True (first chunk) runs the contraction dim as the
                outer loop across all 8 Q/K PSUM tiles, so the PE starts as
                soon as embed-block 0 of x and Wq/Wk has landed instead of
                waiting for the whole chunk's DMAs."""
                qt, kt = [], []
                if c_outer:
                    pq = [
                        ps.tile([128, NBS], F32, tag="ps", name=f"pq{h}")
                        for h in range(H)
                    ]
                    pk = [
                        ps.tile([128, NBS], F32, tag="ps", name=f"pk{h}")
                        for h in range(H)
                    ]
                    for c in range(EC):
                        for h in range(H):
                            nc.tensor.matmul(
                                pq[h],
                                w_sb["q"][:, c, h * 128 : (h + 1) * 128],
                                xt[:, c],
                                start=(c == 0),
                                stop=(c == EC - 1),
                            )
                            nc.tensor.matmul(
                                pk[h],
                                w_sb["k"][:, c, h * 128 : (h + 1) * 128],
                                xt[:, c],
                                start=(c == 0),
                                stop=(c == EC - 1),
                            )
                    # Split the 8 copies across ACT/DVE: iteration 0 has no
                    # projection block ahead to hide a serial ACT chain, and
                    # scores(0) + exp(0) wait on these.
                    for h in range(H):
                        t = qkv.tile([128, NBS], F16, tag=f"qt{h}")
                        nc.scalar.add(out=t, in_=pq[h], add=bq_sb[:, h : h + 1])
                        qt.append(t)
                        t = qkv.tile([128, NBS], F16, tag=f"kt{h}")
                        nc.vector.tensor_scalar_add(
                            out=t, in0=pk[h], scalar1=bk_sb[:, h : h + 1]
                        )
                        kt.append(t)
                else:
                    for h in range(H):
                        p = ps.tile([128, NBS], F32, tag="ps")
                        for c in range(EC):
                            nc.tensor.matmul(
                                p,
                                w_sb["q"][:, c, h * 128 : (h + 1) * 128],
                                xt[:, c],
                                start=(c == 0),
                                stop=(c == EC - 1),
                            )
                        t = qkv.tile([128, NBS], F16, tag=f"qt{h}")
                        nc.scalar.add(out=t, in_=p, add=bq_sb[:, h : h + 1])
                        qt.append(t)
                        p = ps.tile([128, NBS], F32, tag="ps")
                        for c in range(EC):
                            nc.tensor.matmul(
                                p,
                                w_sb["k"][:, c, h * 128 : (h + 1) * 128],
                                xt[:, c],
                                start=(c == 0),
                                stop=(c == EC - 1),
                            )
                        t = qkv.tile([128, NBS], F16, tag=f"kt{h}")
                        nc.scalar.add(out=t, in_=p, add=bk_sb[:, h : h + 1])
                        kt.append(t)
                v_sb = []
                for j in range(NB):
                    p = ps.tile([128, E], F32, tag="ps")
                    for c in range(EC):
                        nc.tensor.matmul(
                            p,
                            xt[:, c, j * 128 : (j + 1) * 128],
                            w_sb["v"][:, c, :],
                            start=(c == 0),
                            stop=(c == EC - 1),
                        )
                    t = qkv.tile([128, E], F16, tag=f"v{j}")
                    if c_outer:
                        # First chunk: keep ACT free for exp(0) — the next
                        # chunk's K psums WAR on exp's scores reads.
                        nc.vector.tensor_copy(out=t, in_=p)
                    else:
                        nc.scalar.copy(out=t, in_=p)
                    v_sb.append(t)
                return qt, kt, v_sb

            def attn_scores(qt, kt, split=False, after_j=None):
                """scores + softmax (no max-subtraction) -> normalized fp16 w,
                transposed to [t, h, s] via the XBAR. One batched DMA in
                steady state (each dispatch blocks the issuing engine ~1.2us);
                per-batch DMAs for the drain chunk, where att(11,0) starting
                ~2us earlier matters more than total dispatch time."""
                w_bf = wsm.tile([128, NB, H, 128], F16, tag="wbf")
                wt_js = []
                for j in range(NB):
                    ps_s = ps.tile([128, H, 128], F32, tag="ps")
                    for h in range(H):
                        nc.tensor.matmul(
                            ps_s[:, h, :],
                            qt[h][:, j * 128 : (j + 1) * 128],
                            kt[h][:, j * 128 : (j + 1) * 128],
                            start=True,
                            stop=True,
                        )
                    w_exp = wsm.tile([128, H, 128], BF16, tag=f"wexp{j}")
                    nc.scalar.activation(
                        out=w_exp,
                        in_=ps_s,
                        func=mybir.ActivationFunctionType.Exp,
                        bias=0.0,
                        scale=1.0,
                    )
                    sumexp = stats.tile([128, H], F32, tag=f"sumexp{j}")
                    nc.vector.reduce_sum(
                        out=sumexp, in_=w_exp, axis=mybir.AxisListType.X
                    )
                    recip = stats.tile([128, H], F32, tag=f"recip{j}")
                    nc.vector.reciprocal(out=recip, in_=sumexp)
                    for h in range(H):
                        nc.vector.tensor_scalar_mul(
                            out=w_bf[:, j, h, :],
                            in0=w_exp[:, h, :],
                            scalar1=recip[:, h : h + 1],
                        )
                    if split:
                        wt_j = wsm.tile([128, H, 128], F16, tag=f"wtj{j}")
                        nc.sync.dma_start(out=wt_j, in_=w_bf[:, j], transpose=True)
                        wt_js.append(wt_j)
                    if after_j is not None:
                        after_j(j)
                if split:
                    return wt_js
                wt = wsm.tile([128, NB, H, 128], F16, tag="wt")
                nc.sync.dma_start(out=wt, in_=w_bf[:], transpose=True)
                return [wt[:, j] for j in range(NB)]

            def attn_att(wts, v_sb, ats, j, force_scalar=False):
                """attT = v.T-form matmuls + PSUM->SBUF copy for one batch.
                force_scalar puts the copy on ACT (drain mode: DVE is serial
                on the softmax chain and would stall O behind it)."""
                ps_at = ps.tile([128, H, 128], F32, tag="ps")
                for h in range(H):
                    nc.tensor.matmul(
                        ps_at[:, h, :],
                        v_sb[j][:, h * 128 : (h + 1) * 128],
                        wts[j][:, h, :],
                        start=True,
                        stop=True,
                    )
                at = attn.tile([128, H, 128], F16, tag=f"at{j}")
                if force_scalar or j % 2 == 1:
                    nc.scalar.copy(out=at, in_=ps_at)
                else:
                    nc.vector.tensor_copy(out=at, in_=ps_at)
                ats.append(at)

            def attn_o(chunk, ats, j, force_scalar=False):
                """O projection + store for one batch. force_scalar keeps the
                copy off DVE during the drain, where the scheduler would slot
                it into the serial softmax chain and delay the wT transposes."""
                p = ps.tile([128, E], F32, tag="ps")
                for h in range(H):
                    nc.tensor.matmul(
                        p,
                        ats[j][:, h, :],
                        w_sb["o"][:, h, :],
                        start=(h == 0),
                        stop=(h == H - 1),
                    )
                o_sb = attn.tile([128, E], F16, tag=f"o{j}")
                if force_scalar:
                    nc.scalar.copy(out=o_sb, in_=p)
                else:
                    nc.vector.tensor_copy(out=o_sb, in_=p)
                nc.sync.dma_start(out=out[chunk * NB + j], in_=o_sb)

            # Software pipeline, tail delayed one iteration and split around
            # the projection block. Per iteration the PE stream is:
            #   scores(k) | att(k-1) | projections(k+1) | O(k-1)
            # so chunk k's softmax chain (ACT exp -> DVE sum/recip/norm ->
            # XBAR transpose) and chunk k-1's at-copies both drain under the
            # ~10us projection block instead of stalling the PE.
            def tail_block(k):
                ats = ats_all[k] = []
                for j in range(NB):
                    attn_att(wts[k], states[k][2], ats, j)
                for j in range(NB):
                    attn_o(k, ats, j)

            xts = {0: load_trans(0, split=True)}
            load_weight("q", "k")
            xts[1] = load_trans(1) if NCHUNK > 1 else None
            load_biases()
            load_weight("v", "o")
            states = {0: proj(xts[0], c_outer=True)}
            wts = {}
            ats_all = {}
            for k in range(NCHUNK):
                last = k == NCHUNK - 1
                if last and k >= 1:
                    # Drain iteration: interleave att(k-1, j) + an ACT-side
                    # at-copy right after each scores-chain(k, j), so exp(0)
                    # still runs first on ACT, at(k-1,0) lands ~1.3us earlier
                    # (unstalling O(k-1)), and the per-j wT transposes go out
                    # as each batch is normalized.
                    ats = ats_all[k - 1] = []
                    wts[k] = attn_scores(
                        states[k][0],
                        states[k][1],
                        split=True,
                        after_j=lambda j: attn_att(
                            wts[k - 1], states[k - 1][2], ats, j, force_scalar=True
                        ),
                    )
                else:
                    wts[k] = attn_scores(states[k][0], states[k][1])
                    if k >= 1:
                        ats = ats_all[k - 1] = []
                        for j in range(NB):
                            attn_att(wts[k - 1], states[k - 1][2], ats, j)
                if k + 2 < NCHUNK:
                    xts[k + 2] = load_trans(k + 2)
                if k + 1 < NCHUNK:
                    states[k + 1] = proj(xts[k + 1])
                if k >= 1:
                    for j in range(NB):
                        attn_o(k - 1, ats_all[k - 1], j, force_scalar=last)
            tail_block(NCHUNK - 1)

    nc.compile()
    return nc


def make_in_maps(inputs):
    x16 = np.asarray(inputs["x"]).astype(np.float16)
    shared = {
        k: np.ascontiguousarray(np.asarray(inputs[k]).astype(np.float16))
        for k in ("Wq", "Wk", "Wv", "Wo")
    }
    for k in ("bq", "bk"):
        shared[k] = np.ascontiguousarray(np.asarray(inputs[k], dtype=np.float32))
    in_maps = []
    for i in range(NCORES):
        xc = x16[i * BLOC : (i + 1) * BLOC]
        # [k, c, e, (j s)] <- [k*NB+j, s, c*128+e]
        xt = np.ascontiguousarray(
            xc.reshape(NCHUNK, NB, S, EC, 128).transpose(0, 3, 4, 1, 2)
        ).reshape(NCHUNK, EC, 128, NBS)
        in_maps.append({"xt": xt, **shared})
    return in_maps


def kernel(**inputs):
    if "nc" not in _CACHE:
        _CACHE["nc"] = build()
    nc = _CACHE["nc"]

    in_maps = make_in_maps(inputs)
    res = run_bass_kernel_spmd(nc, in_maps, core_ids=list(range(NCORES)))
    o = np.concatenate(
        [res.results[i]["out"].astype(np.float32) for i in range(NCORES)], axis=0
    )
    # bv/bo commute through the softmax (rows sum to 1): fold into one
    # output-side bias applied on the host.
    bias = np.asarray(inputs["bv"], dtype=np.float32) @ np.asarray(
        inputs["Wo"], dtype=np.float32
    ) + np.asarray(inputs["bo"], dtype=np.float32)
    return o + bias


# revision 45
# speedup vs baseline: 1.0573x; 1.0225x over previous
"""Multi-head attention (B=384, S=128, E=512, H=4, D=128) on 8 TRN2 NeuronCores.

Data-parallel: batch 384 -> 48 per core, projection weights replicated.

All matmuls run in fp16 (1 cyc/row on the PE at any moving-dim size, vs
fp32r's 4 cyc/row below 256; fp16's 11 mantissa bits keep the softmax
argument error ~8x below bf16's, rel err ~2e-3 vs the 2e-2 gate). PSUM
accumulation stays fp32. The only bf16 tensor is exp(S): scores reach ~60
and there is no max-subtraction, so e^60 needs fp32/bf16 exponent range;
the normalized weights are back in [0,1] and stored fp16.

Transposes never touch the PE:
  xT  is built on the HOST (numpy repack into [chunk, c, e, (j s)] fp16),
      so x loads are one plain contiguous DMA per chunk.
  w^T rides the DMA XBAR (16x128 tiles), one batched SBUF->SBUF
      dma_start(transpose=True) per chunk.

Biases: bq/bk are per-partition adds fused into the PSUM->SBUF copies on
ACT. bv/bo commute through the softmax (rows sum to 1):
  out = att @ Wo + (bv @ Wo + bo),
so the device computes plain att @ Wo and the host adds the combined bias
to the gathered output.

Per-core dataflow per 4-batch chunk (rows = 4*128 = 512):
  QT   = Wq^T @ xT + bq           [e_out, rows]   (lhsT = Wq chunk)
  KT   = Wk^T @ xT + bk
  V    = x @ Wv                   [rows, e_out]   (lhsT = xT chunk)
  per batch (4 heads packed in the PSUM free dim):
    S    = qT.T @ kT              [S, H, T] scores in PSUM
    w    = exp(S)                 bf16 (ACT)
    wn   = w * (1/rowsum)         fp16 (DVE reduce+recip+scale)
  wT   = dma_transpose(wn, all 4 batches at once)
  attT = lhsT(v) @ wT             [D, H, S]
  O    = att @ Wo -> fp16, DMA out

PE work per chunk: QKVO 32768 cyc + scores 2048 + att 2048 = 36864 cyc
-> 184.3us total at 2.4GHz, the 16-bit floor for this op mix. PSUM->SBUF
copies split ACT (qt/kt/exp/v + 2 at) / DVE (o/reduce/recip/norm + 2 at),
~8.5us per chunk each.

Software pipeline per iteration k (tail delayed one chunk, split around
the projections): scores(k) | att(k-1) | proj(k+1) | O(k-1), so the
softmax chain and at-copies drain under the ~10us projection block.
Startup: 9 dummy matmuls warm the HAM clock-gate while chunk 0 and the
q/k weights stream in (per-block DMAs, c-outer accumulation, sync+scalar
HWDGE queues in parallel); drain: per-batch wT transposes let att(11)
start before the whole last chunk is normalized.

Measured on the 8-core axon pod: ~213-215us HW exec (fast clock state;
the device bimodally throttles ~17% on some runs), rel err 1.72e-3 vs
the 2e-2 gate. Engine busy at 213us wall: PE 90% (193us, vs 184.3
floor), ACT ~62%, DVE ~52%, preamble 7.7us fixed.
"""

import numpy as np

import concourse.bass as bass
import concourse.tile as tile
import concourse.mybir as mybir
from concourse import bacc
from concourse.bass_utils import run_bass_kernel_spmd

B, S, E, H, D = 384, 128, 512, 4, 128
NCORES = 8
BLOC = B // NCORES  # 48 batches per core
NB = 4  # batches per chunk
NCHUNK = BLOC // NB
NBS = NB * S  # 512 rows of x per chunk
EC = E // 128  # 4 chunks of the embed dim

F32 = mybir.dt.float32
F16 = mybir.dt.float16
BF16 = mybir.dt.bfloat16

_CACHE = {}


def build():
    nc = bacc.Bacc("TRN2", target_bir_lowering=False, debug=False, num_devices=NCORES)

    # x arrives host-pretransposed: xt[k, c, e, j*128+s] = x[k*NB+j, s, c*128+e]
    xt_dram = nc.dram_tensor("xt", [NCHUNK, EC, 128, NBS], F16, kind="ExternalInput").ap()
    wq = nc.dram_tensor("Wq", [E, E], F16, kind="ExternalInput").ap()
    wk = nc.dram_tensor("Wk", [E, E], F16, kind="ExternalInput").ap()
    wv = nc.dram_tensor("Wv", [E, E], F16, kind="ExternalInput").ap()
    wo = nc.dram_tensor("Wo", [E, E], F16, kind="ExternalInput").ap()
    bq = nc.dram_tensor("bq", [E], F32, kind="ExternalInput").ap()
    bk = nc.dram_tensor("bk", [E], F32, kind="ExternalInput").ap()
    out = nc.dram_tensor("out", [BLOC, S, E], F16, kind="ExternalOutput").ap()

    with tile.TileContext(nc) as tc:
        with (
            tc.tile_pool(name="singles", bufs=1) as singles,
            tc.tile_pool(name="xp", bufs=3) as xp,
            tc.tile_pool(name="qkv", bufs=3) as qkv,
            tc.tile_pool(name="attn", bufs=2) as attn,
            tc.tile_pool(name="wsm", bufs=3) as wsm,
            tc.tile_pool(name="stats", bufs=4) as stats,
            tc.tile_pool(name="ps", bufs=8, space="PSUM") as ps,
        ):
            # --- weights / biases ---
            w_sb = {}
            w_dram = {"q": wq, "k": wk, "v": wv, "o": wo}
            for name in ("q", "k", "v", "o"):
                w_sb[name] = singles.tile([128, EC, E], F16, tag=f"w{name}", name=f"w{name}")

            def load_weight(*names):
                # Scalar HWDGE queue: overlaps the sync queue's x loads
                # during the prologue (ACT itself is idle until the first
                # exp, well after these dispatches). Multiple weights are
                # interleaved per embed-block so the first chunk's c-outer
                # Q/K accumulation gets both operands of block c together.
                for c in range(EC):
                    for name in names:
                        nc.scalar.dma_start(
                            out=w_sb[name][:, c, :],
                            in_=w_dram[name][c * 128 : (c + 1) * 128, :],
                        )

            bq_sb = singles.tile([128, EC], F32, tag="bq")
            bk_sb = singles.tile([128, EC], F32, tag="bk")

            def load_biases():
                for t, b in ((bq_sb, bq), (bk_sb, bk)):
                    nc.scalar.dma_start(
                        out=t,
                        in_=bass.AP(tensor=b.tensor, offset=0, ap=[[1, 128], [128, EC]]),
                    )

            # Warm the PE HAM clock-gate during the initial DMA window with
            # dummy matmuls (PE would otherwise start cold at half clock).
            dummy = singles.tile([128, E], F16, tag="dummy")
            nc.gpsimd.memset(dummy, 0.0)
            warm_ps = ps.tile([128, E], F32, tag="ps", name="warm")
            for _ in range(9):
                nc.tensor.matmul(warm_ps, dummy[:, :128], dummy, start=True, stop=True)

            def load_trans(chunk, split=False):
                """One contiguous DMA of the host-pretransposed x chunk
                (split per embed-block for the first chunk so the first
                projection matmuls can start on block 0)."""
                xt = xp.tile([128, EC, NBS], F16, tag="xt")
                if split:
                    for c in range(EC):
                        nc.sync.dma_start(out=xt[:, c], in_=xt_dram[chunk, c])
                else:
                    nc.sync.dma_start(
                        out=xt, in_=xt_dram[chunk].rearrange("c e f -> e c f")
                    )
                return xt

            def proj(xt, c_outer=False):
                """QT/KT/V projections from xT.

                c_outer=True (first chunk) runs the contraction dim as the
                outer loop across all 8 Q/K PSUM tiles, so the PE starts as
                soon as embed-block 0 of x and Wq/Wk has landed instead of
                waiting for the whole chunk's DMAs."""
                qt, kt = [], []
                if c_outer:
                    pq = [
                        ps.tile([128, NBS], F32, tag="ps", name=f"pq{h}")
                        for h in range(H)
                    ]
                    pk = [
                        ps.tile([128, NBS], F32, tag="ps", name=f"pk{h}")
                        for h in range(H)
                    ]
                    for c in range(EC):
                        for h in range(H):
                            nc.tensor.matmul(
                                pq[h],
                                w_sb["q"][:, c, h * 128 : (h + 1) * 128],
                                xt[:, c],
                                start=(c == 0),
                                stop=(c == EC - 1),
                            )
                            nc.tensor.matmul(
                                pk[h],
                                w_sb["k"][:, c, h * 128 : (h + 1) * 128],
                                xt[:, c],
                                start=(c == 0),
                                stop=(c == EC - 1),
                            )
                    # Split the 8 copies across ACT/DVE: iteration 0 has no
                    # projection block ahead to hide a serial ACT chain, and
                    # scores(0) + exp(0) wait on these.
                    for h in range(H):
                        t = qkv.tile([128, NBS], F16, tag=f"qt{h}")
                        nc.scalar.add(out=t, in_=pq[h], add=bq_sb[:, h : h + 1])
                        qt.append(t)
                        t = qkv.tile([128, NBS], F16, tag=f"kt{h}")
                        nc.vector.tensor_scalar_add(
                            out=t, in0=pk[h], scalar1=bk_sb[:, h : h + 1]
                        )
                        kt.append(t)
                else:
                    for h in range(H):
                        p = ps.tile([128, NBS], F32, tag="ps")
                        for c in range(EC):
                            nc.tensor.matmul(
                                p,
                                w_sb["q"][:, c, h * 128 : (h + 1) * 128],
                                xt[:, c],
                                start=(c == 0),
                                stop=(c == EC - 1),
                            )
                        t = qkv.tile([128, NBS], F16, tag=f"qt{h}")
                        nc.scalar.add(out=t, in_=p, add=bq_sb[:, h : h + 1])
                        qt.append(t)
                        p = ps.tile([128, NBS], F32, tag="ps")
                        for c in range(EC):
                            nc.tensor.matmul(
                                p,
                                w_sb["k"][:, c, h * 128 : (h + 1) * 128],
                                xt[:, c],
                                start=(c == 0),
                                stop=(c == EC - 1),
                            )
                        t = qkv.tile([128, NBS], F16, tag=f"kt{h}")
                        nc.scalar.add(out=t, in_=p, add=bk_sb[:, h : h + 1])
                        kt.append(t)
                v_sb = []
                for j in range(NB):
                    p = ps.tile([128, E], F32, tag="ps")
                    for c in range(EC):
                        nc.tensor.matmul(
                            p,
                            xt[:, c, j * 128 : (j + 1) * 128],
                            w_sb["v"][:, c, :],
                            start=(c == 0),
                            stop=(c == EC - 1),
                        )
                    t = qkv.tile([128, E], F16, tag=f"v{j}")
                    if c_outer:
                        # First chunk: keep ACT free for exp(0) — the next
                        # chunk's K psums WAR on exp's scores reads.
                        nc.vector.tensor_copy(out=t, in_=p)
                    else:
                        nc.scalar.copy(out=t, in_=p)
                    v_sb.append(t)
                return qt, kt, v_sb

            def attn_scores(qt, kt, split=False, after_j=None):
                """scores + softmax (no max-subtraction) -> normalized fp16 w,
                transposed to [t, h, s] via the XBAR. One batched DMA in
                steady state (each dispatch blocks the issuing engine ~1.2us);
                per-batch DMAs for the drain chunk, where att(11,0) starting
                ~2us earlier matters more than total dispatch time."""
                w_bf = wsm.tile([128, NB, H, 128], F16, tag="wbf")
                wt_js = []
                for j in range(NB):
                    ps_s = ps.tile([128, H, 128], F32, tag="ps")
                    for h in range(H):
                        nc.tensor.matmul(
                            ps_s[:, h, :],
                            qt[h][:, j * 128 : (j + 1) * 128],
                            kt[h][:, j * 128 : (j + 1) * 128],
                            start=True,
                            stop=True,
                        )
                    w_exp = wsm.tile([128, H, 128], BF16, tag=f"wexp{j}")
                    nc.scalar.activation(
                        out=w_exp,
                        in_=ps_s,
                        func=mybir.ActivationFunctionType.Exp,
                        bias=0.0,
                        scale=1.0,
                    )
                    sumexp = stats.tile([128, H], F32, tag=f"sumexp{j}")
                    nc.vector.reduce_sum(
                        out=sumexp, in_=w_exp, axis=mybir.AxisListType.X
                    )
                    recip = stats.tile([128, H], F32, tag=f"recip{j}")
                    nc.vector.reciprocal(out=recip, in_=sumexp)
                    for h in range(H):
                        nc.vector.tensor_scalar_mul(
                            out=w_bf[:, j, h, :],
                            in0=w_exp[:, h, :],
                            scalar1=recip[:, h : h + 1],
                        )
                    if split:
                        wt_j = wsm.tile([128, H, 128], F16, tag=f"wtj{j}")
                        nc.sync.dma_start(out=wt_j, in_=w_bf[:, j], transpose=True)
                        wt_js.append(wt_j)
                    if after_j is not None:
                        after_j(j)
                if split:
                    return wt_js
                wt = wsm.tile([128, NB, H, 128], F16, tag="wt")
                nc.sync.dma_start(out=wt, in_=w_bf[:], transpose=True)
                return [wt[:, j] for j in range(NB)]

            def attn_att(wts, v_sb, ats, j, force_scalar=False):
                """attT = v.T-form matmuls + PSUM->SBUF copy for one batch.
                force_scalar puts the copy on ACT (drain mode: DVE is serial
                on the softmax chain and would stall O behind it)."""
                ps_at = ps.tile([128, H, 128], F32, tag="ps")
                for h in range(H):
                    nc.tensor.matmul(
                        ps_at[:, h, :],
                        v_sb[j][:, h * 128 : (h + 1) * 128],
                        wts[j][:, h, :],
                        start=True,
                        stop=True,
                    )
                at = attn.tile([128, H, 128], F16, tag=f"at{j}")
                if force_scalar or j % 2 == 1:
                    nc.scalar.copy(out=at, in_=ps_at)
                else:
                    nc.vector.tensor_copy(out=at, in_=ps_at)
                ats.append(at)

            def attn_o(chunk, ats, j, force_scalar=False):
                """O projection + store for one batch. force_scalar keeps the
                copy off DVE during the drain, where the scheduler would slot
                it into the serial softmax chain and delay the wT transposes."""
                p = ps.tile([128, E], F32, tag="ps")
                for h in range(H):
                    nc.tensor.matmul(
                        p,
                        ats[j][:, h, :],
                        w_sb["o"][:, h, :],
                        start=(h == 0),
                        stop=(h == H - 1),
                    )
                o_sb = attn.tile([128, E], F16, tag=f"o{j}")
                if force_scalar:
                    nc.scalar.copy(out=o_sb, in_=p)
                else:
                    nc.vector.tensor_copy(out=o_sb, in_=p)
                nc.sync.dma_start(out=out[chunk * NB + j], in_=o_sb)

            # Software pipeline, tail delayed one iteration and split around
            # the projection block. Per iteration the PE stream is:
            #   scores(k) | att(k-1) | projections(k+1) | O(k-1)
            # so chunk k's softmax chain (ACT exp -> DVE sum/recip/norm ->
            # XBAR transpose) and chunk k-1's at-copies both drain under the
            # ~10us projection block instead of stalling the PE.
            def tail_block(k):
                ats = ats_all[k] = []
                for j in range(NB):
                    attn_att(wts[k], states[k][2], ats, j)
                for j in range(NB):
                    attn_o(k, ats, j)

            xts = {0: load_trans(0, split=True)}
            load_weight("q", "k")
            xts[1] = load_trans(1) if NCHUNK > 1 else None
            load_biases()
            load_weight("v", "o")
            states = {0: proj(xts[0], c_outer=True)}
            wts = {}
            ats_all = {}
            for k in range(NCHUNK):
                wts[k] = attn_scores(
                    states[k][0], states[k][1], split=(k == NCHUNK - 1)
                )
                if k >= 1:
                    ats = ats_all[k - 1] = []
                    for j in range(NB):
                        attn_att(wts[k - 1], states[k - 1][2], ats, j)
                if k + 2 < NCHUNK:
                    xts[k + 2] = load_trans(k + 2)
                if k + 1 < NCHUNK:
                    states[k + 1] = proj(xts[k + 1])
                if k >= 1:
                    for j in range(NB):
                        attn_o(k - 1, ats_all[k - 1], j)
            tail_block(NCHUNK - 1)

    nc.compile()
    return nc


def make_in_maps(inputs):
    x16 = np.asarray(inputs["x"]).astype(np.float16)
    shared = {
        k: np.ascontiguousarray(np.asarray(inputs[k]).astype(np.float16))
        for k in ("Wq", "Wk", "Wv", "Wo")
    }
    for k in ("bq", "bk"):
        shared[k] = np.ascontiguousarray(np.asarray(inputs[k], dtype=np.float32))
    in_maps = []
    for i in range(NCORES):
        xc = x16[i * BLOC : (i + 1) * BLOC]
        # [k, c, e, (j s)] <- [k*NB+j, s, c*128+e]
        xt = np.ascontiguousarray(
            xc.reshape(NCHUNK, NB, S, EC, 128).transpose(0, 3, 4, 1, 2)
        ).reshape(NCHUNK, EC, 128, NBS)
        in_maps.append({"xt": xt, **shared})
    return in_maps


def kernel(**inputs):
    if "nc" not in _CACHE:
        _CACHE["nc"] = build()
    nc = _CACHE["nc"]

    in_maps = make_in_maps(inputs)
    res = run_bass_kernel_spmd(nc, in_maps, core_ids=list(range(NCORES)))
    o = np.concatenate(
        [res.results[i]["out"].astype(np.float32) for i in range(NCORES)], axis=0
    )
    # bv/bo commute through the softmax (rows sum to 1): fold into one
    # output-side bias applied on the host.
    bias = np.asarray(inputs["bv"], dtype=np.float32) @ np.asarray(
        inputs["Wo"], dtype=np.float32
    ) + np.asarray(inputs["bo"], dtype=np.float32)
    return o + bias
